# revision 31
# baseline (speedup 1.0000x reference)
"""Trainium2 Bass kernel for nn_DVT_69475390980615 (dense_transformer).

Sharding: 8 cores = 4 batches x 2 head-halves. Core c handles batch c//2
and heads [4*(c%2), 4*(c%2)+4).

Host-side folds (numpy, cheap):
  - BN scale folded into conv weights; SCALE folded into Wq.
  - Branch-1 (learned position logits): A = diag(gmk) @ Wmk @ diag(gq) @ Wq
    so dots1^T = (A @ x)^T computed directly from x; A is pre-scaled by
    128/ln2 so the matmul output is already in the Schraudolph integer
    domain.
  - Relative-position bias shipped as EBL = round((bias-headmax)*128/ln2)
    + 16256 in int16: on device one DVE tensor_add of (pd1 + EBL) written
    through an int16 view of a bf16 tile IS exp(dots1+bias) (log-domain
    Schraudolph exp, ~2% rms).  Branch-2 keeps the exact ACT exp.
  - gelu via tanh approximation; its 0.5 factor folded into Wo.

Device dataflow (per core, per head, per j-tile):
  dots2 matmul -> ACT exp -> PV   |   A@x matmul -> DVE add(EBL) -> PV.
  Per-iteration PE order: dots2, PV-D(prev), A@x, PV-B(prev) so the PSUM
  slot rotation never stalls on the DVE add.  V^T is produced by a
  transposed matmul (stationary = x tile, moving = Wv^T) with bias via a
  K=1 ones-row matmul.  Softmax denominators ride as a ones-column in
  V^T (M=65 PV matmuls).  Normalization (SBUF->SBUF shift DMA of the
  denominator rows, recip, DRAM-broadcast, mul) and the gelu chain are
  software-pipelined one head behind.  The gelu output is packed onto
  128 partitions (branch-2 rows 0-63, branch-1 rows 64-127 via one
  shift DMA per head) so the output 1x1 conv runs K=128 matmuls.
  DMA queues: sync=x/carry/out, scalar=weights/at, gpsimd=EBL stream.
  Host sums the two half-core partials per batch and applies the final
  affine.
"""

import sys

for _p in ("/opt/trn_rl_repo",):
    if _p not in sys.path:
        sys.path.insert(0, _p)

import numpy as np
from ml_dtypes import bfloat16

FMAP = 32
HEADS = 8
DK = 32
DV = 64
N = FMAP * FMAP  # 1024
DIM = 256
B = 4
SCALE = DK ** -0.5
HPC = 4  # heads per core
N_CORES = 8
NJT = 8  # j tiles of 128

# gelu tanh-approx constants: gelu(x) ~= 0.5*x*(1+tanh(c*(x + a*x^3)))
GELU_A_INV = 1.0 / 0.044715          # 22.363890
GELU_TANH_SCALE = 0.7978845608028654 * 0.044715  # sqrt(2/pi)*a

# Schraudolph bf16 exp: exp(x) ~= bitcast_bf16(i16(x*128/ln2 + 16256))
SCHR_A = 128.0 / float(np.log(2.0))
SCHR_B = 16256.0

_PROGRAM = None


def _pos_idx():
    r = np.arange(FMAP)
    ii, jj = np.meshgrid(r, r, indexing="ij")
    pos = np.stack([ii.reshape(-1), jj.reshape(-1)], axis=-1)  # (N,2)
    rel = np.abs(pos[:, None, :] - pos[None, :, :])  # (N,N,2)
    return rel[..., 0] * FMAP + rel[..., 1]  # (N,N) int


def _build_program():
    import concourse.bass as bass
    import concourse.tile as tile
    from concourse import bacc, mybir
    from concourse.bass import ts
    from contextlib import ExitStack

    f32 = mybir.dt.float32
    bf16 = mybir.dt.bfloat16
    i16 = mybir.dt.int16
    AF = mybir.ActivationFunctionType
    OP = mybir.AluOpType

    nc = bacc.Bacc(
        "TRN2",
        target_bir_lowering=False,
        debug=False,
        enable_asserts=False,
        num_devices=N_CORES,
    )

    x_d = nc.dram_tensor("x", [128, 2, N], bf16, kind="ExternalInput").ap()
    at_d = nc.dram_tensor(
        "at", [128, 2, HPC * N], bf16, kind="ExternalInput"
    ).ap()
    eb_d = nc.dram_tensor("eb", [HPC, NJT, 128, N], i16, kind="ExternalInput").ap()
    wall_d = nc.dram_tensor("wall", [128, 2048], bf16, kind="ExternalInput").ap()
    bqk_d = nc.dram_tensor("bqk", [128, 2], f32, kind="ExternalInput").ap()
    bv_d = nc.dram_tensor("bv", [1, HPC * DV], bf16, kind="ExternalInput").ap()
    out_d = nc.dram_tensor("out", [DIM, N], bf16, kind="ExternalOutput").ap()

    with tile.TileContext(nc) as tc, ExitStack() as ctx:
        const = ctx.enter_context(tc.tile_pool(name="const", bufs=1))

        # ---- persistent loads (sync: x; scalar: weights/biases) ----
        xs = const.tile([128, 2, N], bf16)
        nc.sync.dma_start(xs[:, 0], x_d[:, 0])
        nc.sync.dma_start(xs[:, 1], x_d[:, 1])
        wall = const.tile([128, 2048], bf16)
        nc.scalar.dma_start(wall[:], wall_d)
        wqv = wall[:, 0:256].rearrange("p (t m) -> p t m", t=2)
        wkv = wall[:, 256:512].rearrange("p (t m) -> p t m", t=2)
        wvv = wall[:, 512:1024].rearrange("p (t m) -> p t m", t=2)
        wov = wall[:, 1024:2048].rearrange("p (h o m) -> p h o m", h=HPC, o=2)
        bqk = const.tile([128, 2], f32)
        nc.scalar.dma_start(bqk[:], bqk_d)
        bvr = const.tile([1, HPC * DV], bf16)
        nc.scalar.dma_start(bvr[:], bv_d)

        onesr = const.tile([1, 128], bf16)
        nc.vector.memset(onesr[:], 1.0)
        onesf = const.tile([33, 128], f32)
        nc.vector.memset(onesf[:], 1.0)
        qs = const.tile([128, N], bf16)
        ks = const.tile([128, N], bf16)
        vts = const.tile([128, NJT, HPC, DV + 1], bf16)  # [j, jt, h, d|ones]
        nc.vector.memset(vts[:, :, :, DV], 1.0)
        gt = const.tile([128, HPC, N], bf16)  # gelu'd Wo input (D|B stacked)
        ob = const.tile([128, 2, N], bf16)  # final partial output

        with (
            tc.tile_pool(name="pd", bufs=2, space="PSUM") as pdp,
            tc.tile_pool(name="po", bufs=2, space="PSUM") as pop,
            tc.tile_pool(name="apool", bufs=5) as apool,
            tc.tile_pool(name="ebpool", bufs=6) as ebpool,
            tc.tile_pool(name="athp", bufs=2) as athp,
            tc.tile_pool(name="ubp", bufs=4) as ubp,
            tc.tile_pool(name="scp", bufs=4) as scp,
            tc.tile_pool(name="rbp", bufs=2) as rbp,
            tc.tile_pool(name="drsc", bufs=2, space="DRAM") as drsc,
            tc.tile_pool(name="gbp", bufs=2) as gbp,
            tc.tile_pool(name="tp", bufs=3) as tp,
        ):
            # ---- phase 0: HAM warmup during the input-DMA wait ----
            dum = pop.tile([128, N], f32, tag="po", name="dum")
            for w in range(12):
                nc.tensor.matmul(
                    dum[:, 0:512], onesr[:], qs[0:1, 0:512],
                    start=True, stop=True,
                )
            nc.scalar.activation(qs[0:1, 0:8], dum[0:1, 0:8], AF.Copy)

            # ---- phase 1: QK projections ----
            pq = pdp.tile([128, N], f32, tag="pd")
            for kt in range(2):
                for isl in range(2):
                    nc.tensor.matmul(
                        pq[:, ts(isl, 512)], wqv[:, kt], xs[:, kt, ts(isl, 512)],
                        start=kt == 0, stop=kt == 1,
                    )
            nc.scalar.activation(qs[:], pq[:], AF.Identity, bias=bqk[:, 0:1])
            pk = pdp.tile([128, N], f32, tag="pd")
            for kt in range(2):
                for isl in range(2):
                    nc.tensor.matmul(
                        pk[:, ts(isl, 512)], wkv[:, kt], xs[:, kt, ts(isl, 512)],
                        start=kt == 0, stop=kt == 1,
                    )
            nc.scalar.activation(ks[:], pk[:], AF.Identity, bias=bqk[:, 1:2])

            # ---- phase 2: V^T via transposed matmul (out rows = j) ----
            for jt in range(NJT):
                pvt = pop.tile([128, N], f32, tag="po")
                for kt in range(2):
                    nc.tensor.matmul(
                        pvt[:, 0 : HPC * DV],
                        xs[:, kt, ts(jt, 128)], wvv[:, kt],
                        start=kt == 0, stop=False,
                    )
                nc.tensor.matmul(
                    pvt[:, 0 : HPC * DV], onesr[:], bvr[:],
                    start=False, stop=True,
                )
                nc.scalar.activation(
                    vts[:, jt, :, 0:DV], pvt[:, 0 : HPC * DV], AF.Copy
                )

            # ---- early prefetches ----
            eb_tiles = {}

            def issue_eb(k):
                if k >= HPC * NJT:
                    return
                h, jt = divmod(k, NJT)
                t = ebpool.tile([128, N], i16, tag="eb")
                # split the 8.4MB bias stream across two hardware DMA
                # queues (gpsimd=q0, scalar=q10) - one queue alone paces
                # the whole loop at ~2.4us/tile
                if k % 2 == 0:
                    nc.gpsimd.dma_start(t[:], eb_d[h, jt])
                else:
                    nc.scalar.dma_start(t[:], eb_d[h, jt])
                eb_tiles[k] = t

            def issue_ath(h):
                t = athp.tile([128, 2, N], bf16, tag="ath")
                nc.scalar.dma_start(t[:], at_d[:, :, h * N : (h + 1) * N])
                return t

            for k in range(5):
                issue_eb(k)
            ath0 = issue_ath(0)

            # ---- phase 3: attention ----
            ath = ath0
            ath_next = None
            pend = None  # deferred PV emission: one j-tile behind

            def emit_pv_D(jt, attn2, attn1, vv, pD, pB):
                for isl in range(2):
                    nc.tensor.matmul(
                        pD[0 : DV + 1, ts(isl, 512)], vv, attn2[:, ts(isl, 512)],
                        start=jt == 0, stop=jt == NJT - 1,
                    )

            def emit_pv_B(jt, attn2, attn1, vv, pD, pB):
                for isl in range(2):
                    nc.tensor.matmul(
                        pB[0 : DV + 1, ts(isl, 512)], vv, attn1[:, ts(isl, 512)],
                        start=jt == 0, stop=jt == NJT - 1,
                    )

            # carry state for the pipelined normalize+gelu of head h-1
            prev = None   # dict with ubD, ubB tiles of previous head
            pending_inner = None  # inner of head h-2 awaiting tanh/final

            for h in range(HPC):
                poD = pop.tile([128, N], f32, tag="po", name=f"poD_{h}")
                poB = pop.tile([128, N], f32, tag="po", name=f"poB_{h}")
                carry = {}
                for jt in range(NJT):
                    issue_eb(h * NJT + jt + 5)
                    ebt = eb_tiles.pop(h * NJT + jt)

                    # alternate slot order so pd1_{k+1} lands on the slot
                    # freed by the fast exp_k, not by the slow TT-add_k
                    if jt % 2 == 0:
                        pd2 = pdp.tile([128, N], f32, tag="pd")
                        pd1 = pdp.tile([128, N], f32, tag="pd")
                    else:
                        pd1 = pdp.tile([128, N], f32, tag="pd")
                        pd2 = pdp.tile([128, N], f32, tag="pd")
                    for isl in range(2):
                        nc.tensor.matmul(
                            pd2[:, ts(isl, 512)],
                            ks[h * DK : (h + 1) * DK, ts(jt, 128)],
                            qs[h * DK : (h + 1) * DK, ts(isl, 512)],
                            start=True, stop=True,
                            tile_position=(h * DK, 0),
                        )
                    attn2 = apool.tile([128, N], bf16, tag="attn")
                    nc.scalar.activation(attn2[:], pd2[:], AF.Exp)
                    for kt in range(2):
                        for isl in range(2):
                            nc.tensor.matmul(
                                pd1[:, ts(isl, 512)],
                                ath[:, kt, ts(jt, 128)],
                                xs[:, kt, ts(isl, 512)],
                                start=kt == 0, stop=kt == 1,
                            )
                    # attn1 = exp(dots1 + bias): log-domain Schraudolph add
                    attn1 = apool.tile([128, N], bf16, tag="attn")
                    nc.vector.tensor_add(attn1[:].bitcast(i16), pd1[:], ebt[:])
                    if pend is not None:
                        emit_pv_D(*pend)
                        emit_pv_B(*pend)

                    pend = (jt, attn2, attn1, vts[:, jt, h], poD, poB)

                    # pipelined carry work of head h-1
                    if prev is not None:
                        if jt == 0:
                            ubD = ubp.tile([128, N], f32, tag="ubD")
                            nc.scalar.activation(
                                ubD[0 : DV + 1, :],
                                prev["poD"][0 : DV + 1, :], AF.Copy,
                            )
                            ubB = ubp.tile([DV + 1, N], f32, tag="ubB")
                            nc.scalar.activation(
                                ubB[:], prev["poB"][0 : DV + 1, :], AF.Copy
                            )
                            carry["ubD"], carry["ubB"] = ubD, ubB
                        elif jt == 1 and pending_inner is not None:
                            th = tp.tile([128, N], bf16, tag="tp")
                            nc.scalar.activation(
                                th[:], pending_inner["ap"][:], AF.Tanh,
                                scale=GELU_TANH_SCALE,
                            )
                            carry["th"] = th
                        elif jt == 2 and pending_inner is not None:
                            nc.vector.scalar_tensor_tensor(
                                gt[:, h - 2, :], carry["th"][:], 1.0,
                                pending_inner["gb"][:], OP.add, OP.mult,
                            )
                        elif jt == 3:
                            scs = scp.tile([2, N], f32, tag="scs")
                            nc.sync.dma_start(
                                scs[0:1, :], carry["ubD"][DV : DV + 1, :]
                            )
                            nc.sync.dma_start(
                                scs[1:2, :], carry["ubB"][DV : DV + 1, :]
                            )
                            carry["scs"] = scs
                        elif jt == 4:
                            rcs = scp.tile([2, N], f32, tag="rcs")
                            nc.vector.reciprocal_approx_fast(
                                out=rcs[:], in_=carry["scs"][:]
                            )
                            drc = drsc.tile([2, N], f32, tag="drc")
                            nc.sync.dma_start(drc[:], rcs[:])
                            carry["drc"] = drc
                        elif jt == 5:
                            rb = rbp.tile([128, N], f32, tag="rb")
                            nc.sync.dma_start(
                                rb[0:DV, :],
                                carry["drc"][0:1, :].to_broadcast((DV, N)),
                            )
                            nc.scalar.dma_start(
                                rb[DV:128, :],
                                carry["drc"][1:2, :].to_broadcast((DV, N)),
                            )
                            carry["rb"] = rb
                            # shift branch-1 numerators into ubD rows 64+
                            nc.sync.dma_start(
                                carry["ubD"][64:128, :],
                                carry["ubB"][0:DV, :],
                            )
                            ath_next = issue_ath(h + 1) if h + 1 < HPC else None
                        elif jt == 6:
                            gb = gbp.tile([128, N], bf16, tag="gb")
                            nc.vector.tensor_mul(
                                gb[:], carry["ubD"][:], carry["rb"][:]
                            )
                            carry["gb"] = gb
                        elif jt == 7:
                            gb = carry["gb"]
                            x2 = tp.tile([128, N], bf16, tag="tp")
                            nc.scalar.activation(x2[:], gb[:], AF.Square)
                            inner = tp.tile([128, N], bf16, tag="tp")
                            nc.vector.scalar_tensor_tensor(
                                inner[:], x2[:], GELU_A_INV, gb[:],
                                OP.add, OP.mult,
                            )
                            carry["inner"] = {"ap": inner, "gb": gb}
                    elif jt == 5 and h == 0:
                        ath_next = issue_ath(1)

                # flush deferred PV of jt7 (branch B first: longer chain)
                jt_, attn2_, attn1_, vv_, pD_, pB_ = pend
                emit_pv_B(jt_, attn2_, attn1_, vv_, pD_, pB_)
                emit_pv_D(jt_, attn2_, attn1_, vv_, pD_, pB_)
                pend = None
                pending_inner = carry.get("inner")  # head h-1's inner
                prev = {"poD": poD, "poB": poB}
                ath = ath_next

            # ---- tail ----
            # head-3 numerator copies (free the po banks first)
            ubD = ubp.tile([128, N], f32, tag="ubD")
            nc.scalar.activation(
                ubD[0 : DV + 1, :], prev["poD"][0 : DV + 1, :], AF.Copy
            )
            ubB = ubp.tile([DV + 1, N], f32, tag="ubB")
            nc.vector.tensor_copy(out=ubB[:], in_=prev["poB"][0 : DV + 1, :])
            # finish head 2 gelu (pending_inner is head 2's)
            p2 = pending_inner
            th = tp.tile([128, N], bf16, tag="tp")
            nc.scalar.activation(th[:], p2["ap"][:], AF.Tanh, scale=GELU_TANH_SCALE)
            nc.vector.scalar_tensor_tensor(
                gt[:, HPC - 2, :], th[:], 1.0, p2["gb"][:], OP.add, OP.mult
            )

            # head-3 normalize chain (issue early, overlaps Wo h0-2)
            # den rows at partitions 0 and 32 (matmul-legal bases)
            scs = scp.tile([33, N], f32, tag="scs3")
            nc.vector.memset(scs[:], 1.0)
            nc.sync.dma_start(scs[32:33, :], ubB[DV : DV + 1, :])
            nc.sync.dma_start(scs[0:1, :], ubD[DV : DV + 1, :])
            nc.sync.dma_start(ubD[64:128, :], ubB[0:DV, :])
            rcs = scp.tile([33, N], f32, tag="rcs3")
            nc.vector.reciprocal_approx_fast(out=rcs[:], in_=scs[:])

            # Wo for heads 0-2 (pw tiles reuse pd slots, free early)
            pws = {}
            for ot in range(2):
                pw = pdp.tile([128, N], f32, tag="pd", name=f"pw_{ot}")
                pws[ot] = pw
                for g in range(HPC - 1):
                    for isl in range(2):
                        nc.tensor.matmul(
                            pw[:, ts(isl, 512)],
                            wov[:, g, ot], gt[:, g, ts(isl, 512)],
                            start=g == 0, stop=False,
                        )

            # rb broadcast via K=1 outer-product matmuls (PSUM, no DMA hop)
            prb = pop.tile([128, N], f32, tag="po", name="prb")
            for isl in range(2):
                nc.tensor.matmul(
                    prb[0:DV, ts(isl, 512)], onesf[0:1, 0:DV],
                    rcs[0:1, ts(isl, 512)], start=True, stop=True,
                    tile_position=(0, 0),
                )
                nc.tensor.matmul(
                    prb[DV:128, ts(isl, 512)], onesf[32:33, 0:DV],
                    rcs[32:33, ts(isl, 512)], start=True, stop=True,
                    tile_position=(32, DV),
                )

            gb = gbp.tile([128, N], bf16, tag="gb")
            nc.vector.tensor_mul(gb[:], ubD[:], prb[:])
            x2 = tp.tile([128, N], bf16, tag="tp")
            nc.scalar.activation(x2[:], gb[:], AF.Square)
            inner = tp.tile([128, N], bf16, tag="tp")
            nc.vector.scalar_tensor_tensor(
                inner[:], x2[:], GELU_A_INV, gb[:], OP.add, OP.mult
            )
            th = tp.tile([128, N], bf16, tag="tp")
            nc.scalar.activation(th[:], inner[:], AF.Tanh, scale=GELU_TANH_SCALE)
            nc.vector.scalar_tensor_tensor(
                gt[:, HPC - 1, :], th[:], 1.0, gb[:], OP.add, OP.mult
            )

            g = HPC - 1
            out_v = out_d.rearrange("(t p) i -> p t i", p=128)
            for ot in range(2):
                for isl in range(2):
                    nc.tensor.matmul(
                        pws[ot][:, ts(isl, 512)],
                        wov[:, g, ot], gt[:, g, ts(isl, 512)],
                        start=False, stop=True,
                    )
                if ot == 0:
                    for isl in range(2):
                        nc.scalar.activation(
                            ob[:, ot, ts(isl, 512)],
                            pws[ot][:, ts(isl, 512)], AF.Copy,
                        )
                    nc.sync.dma_start(out_v[:, ot, :], ob[:, ot, :])
                else:
                    nc.vector.tensor_copy(
                        out=ob[:, ot, :], in_=pws[ot][:, 0:N]
                    )
                    nc.scalar.dma_start(out_v[:, ot, :], ob[:, ot, :])

    nc.compile()
    return nc


def _prepare_in_maps(inputs):
    x = np.asarray(inputs["x"], np.float32)
    Wq = np.asarray(inputs["Wq"], np.float32)
    gq = np.asarray(inputs["gq"], np.float32)
    bq = np.asarray(inputs["bq"], np.float32)
    Wk = np.asarray(inputs["Wk"], np.float32)
    gk = np.asarray(inputs["gk"], np.float32)
    bk = np.asarray(inputs["bk"], np.float32)
    Wv = np.asarray(inputs["Wv"], np.float32)
    gv = np.asarray(inputs["gv"], np.float32)
    bv = np.asarray(inputs["bv"], np.float32)
    Wmk = np.asarray(inputs["Wmk"], np.float32)
    gmk = np.asarray(inputs["gmk"], np.float32)
    bmk = np.asarray(inputs["bmk"], np.float32)
    pos_emb = np.asarray(inputs["pos_emb"], np.float32)
    Wo = np.asarray(inputs["Wo"], np.float32)

    # BN folds
    Wq_f = gq[:, None] * Wq            # unscaled (for branch 1 fold)
    Wq_s = Wq_f * SCALE                # scaled (branch 2 q)
    bq_s = bq * SCALE
    Wk_f = gk[:, None] * Wk
    Wv_f = gv[:, None] * Wv

    # branch-1 fused matrix (pre-scaled into the Schraudolph domain)
    A = (gmk[:, None] * Wmk) @ Wq_f    # (H*N, DIM)
    c1 = gmk * (Wmk @ bq) + bmk        # (H*N,)

    # position bias in the Schraudolph integer domain
    idx = _pos_idx()
    Ball = pos_emb[idx].astype(np.float64) / SCALE   # (N, N, H)
    Ball = np.ascontiguousarray(np.transpose(Ball, (2, 0, 1)))  # (H, j, i)
    Ball += c1.reshape(HEADS, N, 1)
    hmax = Ball.max(axis=(1, 2), keepdims=True)      # (H, 1, 1)
    EBL = np.clip(
        np.rint(SCHR_A * (Ball - hmax)) + SCHR_B, 3000.0, 32767.0
    ).astype(np.int16)                               # (H, j, i)

    x2 = x.reshape(B, DIM, N)

    def pt(a):
        # [(t p), m] -> [p, (t m)] layout used by the on-chip tiles
        m = a.shape[1]
        return a.reshape(2, 128, m).transpose(1, 0, 2).reshape(128, 2 * m)

    in_maps = []
    for core in range(N_CORES):
        b = core // 2
        half = core % 2
        hs = half * HPC
        qrows = slice(hs * DK, (hs + HPC) * DK)
        vrows = slice(hs * DV, (hs + HPC) * DV)
        arows = slice(hs * N, (hs + HPC) * N)

        # wot[h, ot, kk, m]: kk<64 -> branch-2 (dots) channels (Wo col
        # 512 + (hs+h)*64 + kk); kk>=64 -> branch-1 ((hs+h)*64 + kk-64)
        wot = np.empty((HPC, 2, 128, 128), np.float32)
        for h in range(HPC):
            cD = HEADS * DV + (hs + h) * DV
            cB = (hs + h) * DV
            for ot in range(2):
                blk = Wo[ot * 128 : (ot + 1) * 128]
                wot[h, ot, 0:DV, :] = 0.5 * blk[:, cD : cD + DV].T
                wot[h, ot, DV:128, :] = 0.5 * blk[:, cB : cB + DV].T
        # wall[p, :]: wq (2*128) | wk (2*128) | wv (2*256) | wo (4*2*128)
        wall = np.concatenate(
            [
                pt(Wq_s[qrows].T),                 # [128, 256]
                pt(Wk_f[qrows].T),                 # [128, 256]
                pt(Wv_f[vrows].T),                 # [128, 512]
                wot.transpose(2, 0, 1, 3).reshape(128, 1024),
            ],
            axis=1,
        )

        in_maps.append({
            "x": np.ascontiguousarray(
                x2[b].reshape(2, 128, N).transpose(1, 0, 2)
            ).astype(bfloat16),
            "at": np.ascontiguousarray(
                (SCHR_A * A[arows]).T.reshape(2, 128, HPC * N).transpose(1, 0, 2)
            ).astype(bfloat16),
            "eb": np.ascontiguousarray(
                EBL[hs : hs + HPC].reshape(HPC, NJT, 128, N)
            ),
            "wall": np.ascontiguousarray(wall).astype(bfloat16),
            "bqk": np.ascontiguousarray(
                np.stack([bq_s[qrows], bk[qrows]], axis=1)
            ),
            "bv": np.ascontiguousarray(
                bv[vrows].reshape(1, -1)
            ).astype(bfloat16),
        })
    return in_maps


def get_program():
    global _PROGRAM
    if _PROGRAM is None:
        _PROGRAM = _build_program()
    return _PROGRAM


def run_cores(inputs, **run_kwargs):
    """Compile/run the SPMD program; returns BassKernelResults."""
    from concourse.bass_utils import run_bass_kernel_spmd

    nc = get_program()
    in_maps = _prepare_in_maps(inputs)
    res = run_bass_kernel_spmd(
        nc, in_maps, core_ids=list(range(N_CORES)), **run_kwargs
    )
    return res


def kernel(**inputs):
    bo = np.asarray(inputs["bo"], np.float32)
    go = np.asarray(inputs["go"], np.float32)
    bo2 = np.asarray(inputs["bo2"], np.float32)

    res = run_cores(inputs)

    out = np.empty((B, DIM, N), np.float32)
    cbias = (bo * go + bo2)[:, None]
    for b in range(B):
        p = (
            res.results[2 * b]["out"].astype(np.float32)
            + res.results[2 * b + 1]["out"].astype(np.float32)
        )
        out[b] = p * go[:, None] + cbias
    return out.reshape(B, DIM, FMAP, FMAP)
